# revision 25
# baseline (speedup 1.0000x reference)
"""Causal single-head attention (Q==K source bug faithful) on 8 TRN2 NeuronCores.

Problem: x [4, 4096, 1024], Wk/Wv [1024, 64];
  k = q = x@Wk; scores = q k^T / 8, causal softmax, out = weights @ (x@Wv).

v7 strategy — v5's balanced key-split plus PE row-packing and exp split:
  - 8 cores = 4 batches x 2 parities; core parity r owns half the key
    blocks of each 1024-query chunk (positions (j+r)%2 + {0,2,4,6}).
    Each core emits per-chunk partial softmax accumulators
    [65, 1024] = (V|1)^T exp(S^T); host adds the two partials / divides.
  - Scores matmuls have contraction 64 (head dim) — half the PE rows
    idle. Slots pair TWO key blocks via explicit tile_position row
    tiling ((0,0) and (64,0)): both 128x(<=512) score matmuls stream
    concurrently, halving slot stream time. kt holds K^T duplicated on
    partitions 0-63/64-127 ([Wk|Wk] for peer panels, SBUF->SBUF DMA on
    the idle Scalar HW queue for own panels). Chunk 0 runs unpacked so
    nothing waits on the first duplication.
  - exp: ScalarE (scale=1/8 fused) for 28 slots; the 12 chunk-3
    off-diagonal slots use a DVE Schraudolph fake-exp (f32->int16
    mult-add, bitcast bf16, ~1.8% rms on weights) to split the
    softmax-exp bottleneck across two engines.
  - Diagonal masks: one [128,2,256] table multiply per slot; own-range
    on Pool, peer-range on DVE. Output partial copies on DVE.
  - HAM warm-up: dummy matmuls on a memset scratch tile from ~6us so
    real matmuls start at 2.4 GHz; input DMA spread over all three HW
    queues (gpsimd/sync/scalar), first panels split for an early start.
"""
import numpy as np
import ml_dtypes

import concourse.bass as bass
import concourse.mybir as mybir
from concourse import bacc, tile
from concourse.bass_utils import run_bass_kernel_spmd

F32 = mybir.dt.float32
BF16 = mybir.dt.bfloat16
I16 = mybir.dt.int16
EXP = mybir.ActivationFunctionType.Exp
MULT = mybir.AluOpType.mult
ADD = mybir.AluOpType.add

B, T, C, H = 4, 4096, 1024, 64
NCHI = C // 128          # 8 contraction blocks
NPAN = 8                 # panels (0..3 own, 4..7 peer), 512 rows each
PAN = 512
CHUNK = 1024             # queries per chunk
NCK = T // CHUNK         # 4 chunks

# Schraudolph fake-exp: exp(s/8) ~= bitcast_bf16(int16(s*A + B))
FE_A = 0.125 * 1.4426950408889634 * 128.0
FE_B = 16248.6

# const blob layout (bf16 cols per partition); split into 3 DMA pieces
# interleaved with the first input panel halves for an early start.
CW_WKV = 1024            # [8, 128] own weights  [Wk | Wv]
CW_EYE = 65              # eye at [64:128, 0:64], ones col 64
CW_WKK = 1024            # [8, 128] peer weights [Wk | Wk]
CW_MSK = 1536            # [3, 2, 256] masks: 0=own tril, 1=peer jpar0, 2=peer jpar1
O_EYE = CW_WKV
O_WKK = CW_WKV + CW_EYE
O_MSK = O_WKK + CW_WKK
CSTW = CW_WKV + CW_EYE + CW_WKK + CW_MSK


def build_nc():
    nc = bacc.Bacc("TRN2", target_bir_lowering=False, debug=False, num_devices=8)

    xt_d = nc.declare_dram_parameter("xt", [NPAN, 128, NCHI, PAN], BF16, isOutput=False)
    cst_d = nc.declare_dram_parameter("cst", [128, CSTW], BF16, isOutput=False)
    out_d = nc.declare_dram_parameter("out", [NCK, 65, 1024], F32, isOutput=True)

    with tile.TileContext(nc) as tc:
        with (
            tc.tile_pool(name="const", bufs=1) as const,
            tc.tile_pool(name="xt", bufs=NPAN) as xtp,
            tc.tile_pool(name="vsb", bufs=2) as vsbp,
            tc.tile_pool(name="pt", bufs=6) as ptp,
            tc.tile_pool(name="osb", bufs=2) as osbp,
            tc.tile_pool(name="psA", bufs=2, space="PSUM") as psA,
            tc.tile_pool(name="psO", bufs=2, space="PSUM") as psO,
        ):
            cst = const.tile([128, CSTW], BF16, tag="cst")
            wkv = cst[:, 0:CW_WKV].rearrange("p (a b) -> p a b", a=NCHI)
            eyeb = cst[:, O_EYE:O_EYE + CW_EYE]
            wkk = cst[:, O_WKK:O_WKK + CW_WKK].rearrange("p (a b) -> p a b", a=NCHI)
            msk = cst[:, O_MSK:O_MSK + CW_MSK].rearrange("p (a b c) -> p a b c", a=3, b=2)

            kt = const.tile([128, T], BF16, tag="kt")       # rows 0-63 K^T, 64-127 dup
            vaug = const.tile([128, 16, 128], BF16, tag="vaug")  # V|1|0pad per own kb
            scr = const.tile([128, 512], BF16, tag="scr")   # warm-up zeros

            # HAM warm-up: get the PE clock to 2.4 GHz while input DMAs land.
            nc.gpsimd.memset(scr[:], 0)
            wps = psA.tile([128, 1024], F32, tag="ps", name="warm")
            for _ in range(12):
                nc.tensor.matmul(
                    wps[0:65, 0:512], scr[:, 0:65], scr[:],
                    start=True, stop=True,
                )

            # Input DMA on all three HW queues: G (gpsimd) consts + own
            # panels 0-2, S (sync) peer panels 4-6, Scalar panels 3 + 7.
            # The DMA engines round-robin across ALL pending transfers, so
            # only the first-needed pieces are issued here; later panels are
            # issued mid-program (behind dependency-gated instructions in
            # each engine's FIFO) to give the early panels full bandwidth.
            xts = [
                xtp.tile([128, NCHI, PAN], BF16, tag="xt", name=f"xt{p}")
                for p in range(NPAN)
            ]
            nc.gpsimd.dma_start(cst[:, 0:O_WKK], cst_d[:, 0:O_WKK])
            nc.gpsimd.dma_start(xts[0][:, 0:4, :], xt_d[0][:, 0:4, :])
            nc.gpsimd.dma_start(cst[:, O_WKK:O_MSK], cst_d[:, O_WKK:O_MSK])
            nc.gpsimd.dma_start(xts[0][:, 4:8, :], xt_d[0][:, 4:8, :])
            nc.scalar.dma_start(cst[:, O_MSK:], cst_d[:, O_MSK:])
            nc.sync.dma_start(xts[4][:, 0:4, :], xt_d[4][:, 0:4, :])
            nc.sync.dma_start(xts[4][:, 4:8, :], xt_d[4][:, 4:8, :])

            # vaug: ones col 64, zero cols 65..127 (FWL padding)
            nc.vector.tensor_copy(
                vaug[:, :, 64:65],
                eyeb[:, 64:65].unsqueeze(1).broadcast_to([128, 16, 1]),
            )
            nc.gpsimd.memset(vaug[:, :, 65:128], 0)

            for _ in range(2):
                z = psA.tile([128, 1024], F32, tag="ps", name="z")
                nc.scalar.memzero(z[:])

            def proj_own(p):
                xt = xts[p]
                pj = psA.tile([128, 1024], F32, tag="ps")
                for ci in range(NCHI):
                    nc.tensor.matmul(
                        pj[:, 0:PAN], wkv[:, ci, :], xt[:, ci, :],
                        start=(ci == 0), stop=(ci == NCHI - 1),
                    )
                rng = slice(p * PAN, (p + 1) * PAN)
                nc.vector.tensor_copy(kt[0:64, rng], pj[0:64, 0:PAN])
                # duplicate K^T onto partitions 64-127 for row-packed slots;
                # Scalar's HW DMA queue has only panels 3/7 ahead of it.
                nc.scalar.dma_start(kt[64:128, rng], kt[0:64, rng])
                vsb = vsbp.tile([128, PAN], BF16, tag="vsb")
                nc.vector.tensor_copy(vsb[64:128, :], pj[64:128, 0:PAN])
                v_ps = pj[:, PAN:PAN + 128].bitcast(BF16).rearrange(
                    "p (a b) -> p a b", a=4)
                for tb in range(4):
                    nc.tensor.transpose(
                        v_ps[:, tb, :], vsb[64:128, tb * 128:(tb + 1) * 128],
                        eyeb[64:128, 0:64],
                    )
                nc.vector.tensor_copy(vaug[:, 4 * p:4 * p + 4, 0:64], v_ps[:])

            def proj_peer(p):
                xt = xts[p]
                pj = psA.tile([128, 1024], F32, tag="ps")
                for ci in range(NCHI):
                    nc.tensor.matmul(
                        pj[:, 0:PAN], wkk[:, ci, :], xt[:, ci, :],
                        start=(ci == 0), stop=(ci == NCHI - 1),
                    )
                rng = slice(2048 + (p - 4) * PAN, 2048 + (p - 3) * PAN)
                nc.vector.tensor_copy(kt[:, rng], pj[:, 0:PAN])

            ot = {}       # live chunk accumulators [128, 1024]
            started = {}  # (j, rng) -> bank already started

            def scores_exp(j, kA, kB, c0, rng, diag):
                """Pair-slot front half: two score matmuls (row-packed on
                chunks>0) + exp (ScalarE, or DVE fake-exp on chunk-3
                off-diagonal slots) + diag masks (Pool own / DVE peer)."""
                packed = j != 0
                sp = psA.tile([128, 1024], F32, tag="ps")
                spv = sp.rearrange("p (b c) -> p b c", b=2)
                qb = 2048 * rng + j * PAN
                bA = c0
                bB = c0 + 128 if diag else 0
                lo = kt[0:64, :]
                hi = kt[64:128, :] if packed else lo
                tp = (64, 0) if packed else (0, 0)
                nc.tensor.matmul(
                    spv[:, 0, bA:512],
                    lo[:, kA * 128:(kA + 1) * 128], lo[:, qb + bA:qb + 512],
                    start=True, stop=True, tile_position=(0, 0),
                )
                nc.tensor.matmul(
                    spv[:, 1, bB:512],
                    hi[:, kB * 128:(kB + 1) * 128], hi[:, qb + bB:qb + 512],
                    start=True, stop=True, tile_position=tp,
                )
                pt = ptp.tile([128, 2, 512], BF16, tag="pt")
                # exp split per slot: block A on ScalarE (true exp), block B
                # on DVE (Schraudolph fake-exp) — softmax-exp throughput is
                # the slot-period limiter on a single engine.
                nc.scalar.activation(pt[:, 0, bA:512], spv[:, 0, bA:512],
                                     EXP, scale=0.125)
                nc.vector.tensor_scalar(
                    pt.bitcast(I16)[:, 1, bB:512], spv[:, 1, bB:512],
                    FE_A, FE_B, MULT, ADD,
                )
                if diag:
                    mi = 0 if rng == 0 else 1 + (j % 2)
                    # A: tril/parity zero over its own 128-col block (Pool);
                    # B: same over its block (DVE)
                    nc.gpsimd.tensor_mul(
                        pt[:, 0, c0:c0 + 128], pt[:, 0, c0:c0 + 128],
                        msk[:, mi, 0, 0:128],
                    )
                    nc.vector.tensor_mul(
                        pt[:, 1, c0 + 128:c0 + 256], pt[:, 1, c0 + 128:c0 + 256],
                        msk[:, mi, 1, 128:256],
                    )
                return pt

            def pv(j, kA, kB, pt, c0, rng, diag, last):
                acc = ot[j]
                off = 512 * rng
                bA = c0
                bB = c0 + 128 if diag else 0
                firstA = not started.get((j, rng), False)
                started[(j, rng)] = True
                nc.tensor.matmul(
                    acc[:, off + bA:off + 512], vaug[:, kA, :], pt[:, 0, bA:512],
                    start=firstA, stop=last, skip_group_check=True,
                )
                nc.tensor.matmul(
                    acc[:, off + bB:off + 512], vaug[:, kB, :], pt[:, 1, bB:512],
                    start=False, stop=last, skip_group_check=True,
                )

            def chunk_slots(j):
                s = []
                for jj in range(j):
                    for m in (0, 1):
                        for rng in (0, 1):
                            s.append(("slot", dict(
                                j=j, kA=4 * jj + 2 * m, kB=4 * jj + 2 * m + 1,
                                c0=0, rng=rng, diag=False)))
                for m in (0, 1):
                    for rng in (0, 1):
                        s.append(("slot", dict(
                            j=j, kA=4 * j + 2 * m, kB=4 * j + 2 * m + 1,
                            c0=256 * m, rng=rng, diag=True,
                            last=(m == 1 and rng == 1))))
                return s

            actions = [("proj_own", 0), ("proj_peer", 4)]
            actions += chunk_slots(0) + [("dma1", None), ("out", 0), ("warm", (0, 8))]
            actions += [("proj_own", 1), ("proj_peer", 5), ("dma2", None)]
            actions += chunk_slots(1) + [("out", 1), ("warm", (1, 4))]
            actions += [("proj_own", 2), ("proj_peer", 6),
                        ("proj_own", 3), ("proj_peer", 7)]
            c2, c3 = chunk_slots(2), chunk_slots(3)
            mix = []
            while c2 or c3:
                if c2:
                    mix.append(c2.pop(0))
                if c3:
                    mix.append(c3.pop(0))
                if c3:
                    mix.append(c3.pop(0))
            for a in mix:
                actions.append(a)
                if a[1].get("last") and a[1]["j"] == 2:
                    actions.append(("out", 2))
            actions.append(("out", 3))

            pending = None

            def flush():
                nonlocal pending
                if pending is not None:
                    a, pt = pending
                    pv(a["j"], a["kA"], a["kB"], pt, a["c0"], a["rng"],
                       a["diag"], a.get("last", False))
                    pending = None

            for kind, arg in actions:
                if kind == "proj_own":
                    proj_own(arg)
                elif kind == "proj_peer":
                    proj_peer(arg)
                elif kind == "dma1":
                    # Second DMA wave: gated behind chunk-0's mask ops in
                    # the gpsimd FIFO, so it starts only once the first
                    # panels have landed and been consumed.
                    nc.gpsimd.dma_start(xts[1][:], xt_d[1])
                    nc.gpsimd.dma_start(xts[2][:], xt_d[2])
                elif kind == "dma2":
                    # Third wave, behind chunk-0 exps in the scalar FIFO.
                    nc.scalar.dma_start(xts[3][:], xt_d[3])
                    nc.scalar.dma_start(xts[7][:], xt_d[7])
                elif kind == "warm":
                    # Keep-warm fillers bridge DMA-starved PE windows so the
                    # HAM clock gate stays at 8/8. They write the dead rows
                    # 96-127 of the retired chunk accumulator (never read;
                    # later reuse start=True-clears them).
                    jw, n = arg
                    for _ in range(n):
                        nc.tensor.matmul(
                            ot[jw][96:128, 0:512], scr[:, 0:32], scr[:],
                            start=True, stop=True, tile_position=(0, 96),
                            skip_group_check=True,
                        )
                elif kind == "slot":
                    if arg["j"] not in ot:
                        ot[arg["j"]] = psO.tile([128, 1024], F32, tag="ot", name="ot")
                    pt = scores_exp(arg["j"], arg["kA"], arg["kB"],
                                    arg["c0"], arg["rng"], arg["diag"])
                    flush()
                    pending = (arg, pt)
                else:  # out: partials via an SBUF bounce, split DVE/ScalarE
                    flush()
                    osb = osbp.tile([65, 1024], F32, tag="osb")
                    nc.vector.tensor_copy(osb[:, 0:512], ot[arg][0:65, 0:512])
                    nc.scalar.copy(osb[:, 512:1024], ot[arg][0:65, 512:1024])
                    nc.sync.dma_start(out_d.ap()[arg], osb[:])
                    # peer-panel waves ride the sync FIFO behind this
                    # dependency-gated output DMA
                    if arg == 0:
                        nc.sync.dma_start(xts[5][:], xt_d[5])
                    elif arg == 1:
                        nc.sync.dma_start(xts[6][:], xt_d[6])

    nc.compile()
    return nc


def _own_blocks(j, r):
    """Global 128-row key-block positions (within chunk j) owned by
    parity r, in local order."""
    q = (j + r) % 2
    return [q + 2 * i for i in range(4)]


def make_inputs(x, Wk, Wv):
    """Build the 8 per-core input maps (pure layout work)."""
    bf16 = ml_dtypes.bfloat16
    wkv = np.concatenate([Wk, Wv], axis=1)            # [1024, 128]
    wkv_t = np.ascontiguousarray(
        wkv.reshape(NCHI, 128, 128).transpose(1, 0, 2)
    ).astype(bf16)  # [cp, chi, m]
    wkk = np.concatenate([Wk, Wk], axis=1)
    wkk_t = np.ascontiguousarray(
        wkk.reshape(NCHI, 128, 128).transpose(1, 0, 2)
    ).astype(bf16)

    eyeb = np.zeros((128, 65), dtype=np.float32)
    eyeb[64:128, 0:64] = np.eye(64)
    eyeb[:, 64] = 1.0

    kk = np.arange(128)[:, None]                      # key row within block
    cc = np.arange(256)[None, :]                      # window column

    # msk[*, 0]: own-range diag mask. A: tril then ones; B: zeros then tril.
    gm2 = np.empty((128, 2, 256), dtype=np.float32)
    gm2[:, 0, :] = np.concatenate(
        [(cc[:, 0:128] >= kk), np.ones((128, 128))], axis=1)
    gm2[:, 1, :] = np.concatenate(
        [np.zeros((128, 128)), (cc[:, 0:128] >= kk)], axis=1)

    def peer_mask(qpar):
        pm = np.ones((128, 2, 256), dtype=np.float32)
        pm[:, 1, 0:128] = 0.0  # B's never-written prefix stays inert
        if qpar == 1:
            pm[:, 0, 0:128] = 0.0
            pm[:, 1, :] = 0.0
        return pm

    in_maps = []
    for c in range(8):
        b, r = c % 4, c // 4

        xT = np.ascontiguousarray(x[b].T)             # [1024, 4096]
        xr = xT.reshape(NCHI, 128, T)                 # [chi, cp, t]
        xt = np.empty((NPAN, 128, NCHI, PAN), dtype=bf16)
        for p in range(NPAN):
            j, rr = (p, r) if p < 4 else (p - 4, 1 - r)
            rows = np.concatenate([
                np.arange(j * CHUNK + m * 128, j * CHUNK + m * 128 + 128)
                for m in _own_blocks(j, rr)
            ])
            xt[p] = xr[:, :, rows].transpose(1, 0, 2)

        cst = np.empty((128, CSTW), dtype=bf16)
        cst[:, 0:CW_WKV] = wkv_t.reshape(128, -1)
        cst[:, O_EYE:O_EYE + CW_EYE] = eyeb.astype(bf16)
        cst[:, O_WKK:O_WKK + CW_WKK] = wkk_t.reshape(128, -1)
        mtab = np.stack([gm2, peer_mask(r % 2), peer_mask((1 + r) % 2)], axis=1)
        cst[:, O_MSK:O_MSK + CW_MSK] = mtab.reshape(128, -1).astype(bf16)

        in_maps.append({"xt": xt, "cst": cst})
    return in_maps


_NC = None


def get_nc():
    global _NC
    if _NC is None:
        _NC = build_nc()
    return _NC


def kernel(x, Wk, Wv):
    x = np.asarray(x, dtype=np.float32)
    Wk = np.asarray(Wk, dtype=np.float32)
    Wv = np.asarray(Wv, dtype=np.float32)
    nc = get_nc()
    in_maps = make_inputs(x, Wk, Wv)
    res = run_bass_kernel_spmd(nc, in_maps, list(range(8)))

    out = np.empty((B, T, H), dtype=np.float32)
    for b in range(4):
        p0 = res.results[b]["out"].astype(np.float64)      # parity 0
        p1 = res.results[b + 4]["out"].astype(np.float64)  # parity 1
        for j in range(NCK):
            tot = np.zeros((65, 1024), dtype=np.float64)
            for r, part in ((0, p0[j]), (1, p1[j])):
                blocks = _own_blocks(j, r) + _own_blocks(j, 1 - r)
                glob = np.empty((65, 1024), dtype=np.float64)
                for k, m in enumerate(blocks):
                    glob[:, m * 128:(m + 1) * 128] = part[:, k * 128:(k + 1) * 128]
                tot += glob
            out[b, j * CHUNK:(j + 1) * CHUNK] = (tot[0:64] / tot[64]).T
    return out


# revision 27
# speedup vs baseline: 1.0128x; 1.0128x over previous
"""Causal single-head attention (Q==K source bug faithful) on 8 TRN2 NeuronCores.

Problem: x [4, 4096, 1024], Wk/Wv [1024, 64];
  k = q = x@Wk; scores = q k^T / 8, causal softmax, out = weights @ (x@Wv).

v7 strategy — v5's balanced key-split plus PE row-packing and exp split:
  - 8 cores = 4 batches x 2 parities; core parity r owns half the key
    blocks of each 1024-query chunk (positions (j+r)%2 + {0,2,4,6}).
    Each core emits per-chunk partial softmax accumulators
    [65, 1024] = (V|1)^T exp(S^T); host adds the two partials / divides.
  - Scores matmuls have contraction 64 (head dim) — half the PE rows
    idle. Slots pair TWO key blocks via explicit tile_position row
    tiling ((0,0) and (64,0)): both 128x(<=512) score matmuls stream
    concurrently, halving slot stream time. kt holds K^T duplicated on
    partitions 0-63/64-127 ([Wk|Wk] for peer panels, SBUF->SBUF DMA on
    the idle Scalar HW queue for own panels). Chunk 0 runs unpacked so
    nothing waits on the first duplication.
  - exp: ScalarE (scale=1/8 fused) for 28 slots; the 12 chunk-3
    off-diagonal slots use a DVE Schraudolph fake-exp (f32->int16
    mult-add, bitcast bf16, ~1.8% rms on weights) to split the
    softmax-exp bottleneck across two engines.
  - Diagonal masks: one [128,2,256] table multiply per slot; own-range
    on Pool, peer-range on DVE. Output partial copies on DVE.
  - HAM warm-up: dummy matmuls on a memset scratch tile from ~6us so
    real matmuls start at 2.4 GHz; input DMA spread over all three HW
    queues (gpsimd/sync/scalar), first panels split for an early start.
"""
import numpy as np
import ml_dtypes

import concourse.bass as bass
import concourse.mybir as mybir
from concourse import bacc, tile
from concourse.bass_utils import run_bass_kernel_spmd

F32 = mybir.dt.float32
BF16 = mybir.dt.bfloat16
I16 = mybir.dt.int16
EXP = mybir.ActivationFunctionType.Exp
MULT = mybir.AluOpType.mult
ADD = mybir.AluOpType.add

B, T, C, H = 4, 4096, 1024, 64
NCHI = C // 128          # 8 contraction blocks
NPAN = 8                 # panels (0..3 own, 4..7 peer), 512 rows each
PAN = 512
CHUNK = 1024             # queries per chunk
NCK = T // CHUNK         # 4 chunks

# Schraudolph fake-exp: exp(s/8) ~= bitcast_bf16(int16(s*A + B))
FE_A = 0.125 * 1.4426950408889634 * 128.0
FE_B = 16248.6

# const blob layout (bf16 cols per partition); split into 3 DMA pieces
# interleaved with the first input panel halves for an early start.
CW_WKV = 1024            # [8, 128] own weights  [Wk | Wv]
CW_EYE = 65              # eye at [64:128, 0:64], ones col 64
CW_WKK = 1024            # [8, 128] peer weights [Wk | Wk]
CW_MSK = 1536            # [3, 2, 256] masks: 0=own tril, 1=peer jpar0, 2=peer jpar1
O_EYE = CW_WKV
O_WKK = CW_WKV + CW_EYE
O_MSK = O_WKK + CW_WKK
CSTW = CW_WKV + CW_EYE + CW_WKK + CW_MSK


def build_nc():
    nc = bacc.Bacc("TRN2", target_bir_lowering=False, debug=False, num_devices=8)

    xt_d = nc.declare_dram_parameter("xt", [NPAN, 128, NCHI, PAN], BF16, isOutput=False)
    cst_d = nc.declare_dram_parameter("cst", [128, CSTW], BF16, isOutput=False)
    out_d = nc.declare_dram_parameter("out", [NCK, 65, 1024], F32, isOutput=True)

    with tile.TileContext(nc) as tc:
        with (
            tc.tile_pool(name="const", bufs=1) as const,
            tc.tile_pool(name="xt", bufs=NPAN) as xtp,
            tc.tile_pool(name="vsb", bufs=2) as vsbp,
            tc.tile_pool(name="pt", bufs=6) as ptp,
            tc.tile_pool(name="osb", bufs=2) as osbp,
            tc.tile_pool(name="psA", bufs=2, space="PSUM") as psA,
            tc.tile_pool(name="psO", bufs=2, space="PSUM") as psO,
        ):
            cst = const.tile([128, CSTW], BF16, tag="cst")
            wkv = cst[:, 0:CW_WKV].rearrange("p (a b) -> p a b", a=NCHI)
            eyeb = cst[:, O_EYE:O_EYE + CW_EYE]
            wkk = cst[:, O_WKK:O_WKK + CW_WKK].rearrange("p (a b) -> p a b", a=NCHI)
            msk = cst[:, O_MSK:O_MSK + CW_MSK].rearrange("p (a b c) -> p a b c", a=3, b=2)

            kt = const.tile([128, T], BF16, tag="kt")       # rows 0-63 K^T, 64-127 dup
            vaug = const.tile([128, 16, 128], BF16, tag="vaug")  # V|1|0pad per own kb
            scr = const.tile([128, 512], BF16, tag="scr")   # warm-up zeros

            # HAM warm-up: get the PE clock to 2.4 GHz while input DMAs land.
            nc.gpsimd.memset(scr[:], 0)
            wps = psA.tile([128, 1024], F32, tag="ps", name="warm")
            for _ in range(12):
                nc.tensor.matmul(
                    wps[0:65, 0:512], scr[:, 0:65], scr[:],
                    start=True, stop=True,
                )

            # Input DMA on all three HW queues: G (gpsimd) consts + own
            # panels 0-2, S (sync) peer panels 4-6, Scalar panels 3 + 7.
            # The DMA engines round-robin across ALL pending transfers, so
            # only the first-needed pieces are issued here; later panels are
            # issued mid-program (behind dependency-gated instructions in
            # each engine's FIFO) to give the early panels full bandwidth.
            xts = [
                xtp.tile([128, NCHI, PAN], BF16, tag="xt", name=f"xt{p}")
                for p in range(NPAN)
            ]
            nc.gpsimd.dma_start(cst[:, 0:O_WKK], cst_d[:, 0:O_WKK])
            nc.gpsimd.dma_start(xts[0][:, 0:4, :], xt_d[0][:, 0:4, :])
            nc.gpsimd.dma_start(cst[:, O_WKK:O_MSK], cst_d[:, O_WKK:O_MSK])
            nc.gpsimd.dma_start(xts[0][:, 4:8, :], xt_d[0][:, 4:8, :])
            nc.scalar.dma_start(cst[:, O_MSK:], cst_d[:, O_MSK:])
            nc.sync.dma_start(xts[4][:, 0:4, :], xt_d[4][:, 0:4, :])
            nc.sync.dma_start(xts[4][:, 4:8, :], xt_d[4][:, 4:8, :])

            # vaug: ones col 64, zero cols 65..127 (FWL padding)
            nc.vector.tensor_copy(
                vaug[:, :, 64:65],
                eyeb[:, 64:65].unsqueeze(1).broadcast_to([128, 16, 1]),
            )
            nc.gpsimd.memset(vaug[:, :, 65:128], 0)

            for _ in range(2):
                z = psA.tile([128, 1024], F32, tag="ps", name="z")
                nc.scalar.memzero(z[:])

            def proj_own(p):
                xt = xts[p]
                pj = psA.tile([128, 1024], F32, tag="ps")
                for ci in range(NCHI):
                    nc.tensor.matmul(
                        pj[:, 0:PAN], wkv[:, ci, :], xt[:, ci, :],
                        start=(ci == 0), stop=(ci == NCHI - 1),
                    )
                rng = slice(p * PAN, (p + 1) * PAN)
                nc.vector.tensor_copy(kt[0:64, rng], pj[0:64, 0:PAN])
                # duplicate K^T onto partitions 64-127 for row-packed slots;
                # Scalar's HW DMA queue has only panels 3/7 ahead of it.
                nc.scalar.dma_start(kt[64:128, rng], kt[0:64, rng])
                vsb = vsbp.tile([128, PAN], BF16, tag="vsb")
                nc.vector.tensor_copy(vsb[64:128, :], pj[64:128, 0:PAN])
                v_ps = pj[:, PAN:PAN + 128].bitcast(BF16).rearrange(
                    "p (a b) -> p a b", a=4)
                for tb in range(4):
                    nc.tensor.transpose(
                        v_ps[:, tb, :], vsb[64:128, tb * 128:(tb + 1) * 128],
                        eyeb[64:128, 0:64],
                    )
                nc.vector.tensor_copy(vaug[:, 4 * p:4 * p + 4, 0:64], v_ps[:])

            def proj_peer(p):
                xt = xts[p]
                pj = psA.tile([128, 1024], F32, tag="ps")
                for ci in range(NCHI):
                    nc.tensor.matmul(
                        pj[:, 0:PAN], wkk[:, ci, :], xt[:, ci, :],
                        start=(ci == 0), stop=(ci == NCHI - 1),
                    )
                rng = slice(2048 + (p - 4) * PAN, 2048 + (p - 3) * PAN)
                nc.vector.tensor_copy(kt[:, rng], pj[:, 0:PAN])

            ot = {}       # live chunk accumulators [128, 1024]
            started = {}  # (j, rng) -> bank already started

            def scores_exp(j, kA, kB, c0, rng, diag):
                """Pair-slot front half: two score matmuls (row-packed on
                chunks>0) + exp (ScalarE, or DVE fake-exp on chunk-3
                off-diagonal slots) + diag masks (Pool own / DVE peer)."""
                packed = j != 0
                sp = psA.tile([128, 1024], F32, tag="ps")
                spv = sp.rearrange("p (b c) -> p b c", b=2)
                qb = 2048 * rng + j * PAN
                bA = c0
                bB = c0 + 128 if diag else 0
                lo = kt[0:64, :]
                hi = kt[64:128, :] if packed else lo
                tp = (64, 0) if packed else (0, 0)
                nc.tensor.matmul(
                    spv[:, 0, bA:512],
                    lo[:, kA * 128:(kA + 1) * 128], lo[:, qb + bA:qb + 512],
                    start=True, stop=True, tile_position=(0, 0),
                )
                nc.tensor.matmul(
                    spv[:, 1, bB:512],
                    hi[:, kB * 128:(kB + 1) * 128], hi[:, qb + bB:qb + 512],
                    start=True, stop=True, tile_position=tp,
                )
                pt = ptp.tile([128, 2, 512], BF16, tag="pt")
                # exp split per slot: block A on ScalarE (true exp), block B
                # on DVE (Schraudolph fake-exp) — softmax-exp throughput is
                # the slot-period limiter on a single engine.
                nc.scalar.activation(pt[:, 0, bA:512], spv[:, 0, bA:512],
                                     EXP, scale=0.125)
                nc.vector.tensor_scalar(
                    pt.bitcast(I16)[:, 1, bB:512], spv[:, 1, bB:512],
                    FE_A, FE_B, MULT, ADD,
                )
                if diag:
                    mi = 0 if rng == 0 else 1 + (j % 2)
                    # A: tril/parity zero over its own 128-col block (Pool);
                    # B: same over its block (DVE)
                    nc.gpsimd.tensor_mul(
                        pt[:, 0, c0:c0 + 128], pt[:, 0, c0:c0 + 128],
                        msk[:, mi, 0, 0:128],
                    )
                    nc.vector.tensor_mul(
                        pt[:, 1, c0 + 128:c0 + 256], pt[:, 1, c0 + 128:c0 + 256],
                        msk[:, mi, 1, 128:256],
                    )
                return pt

            def pv(j, kA, kB, pt, c0, rng, diag, last):
                acc = ot[j]
                off = 512 * rng
                bA = c0
                bB = c0 + 128 if diag else 0
                firstA = not started.get((j, rng), False)
                started[(j, rng)] = True
                nc.tensor.matmul(
                    acc[:, off + bA:off + 512], vaug[:, kA, :], pt[:, 0, bA:512],
                    start=firstA, stop=last, skip_group_check=True,
                )
                nc.tensor.matmul(
                    acc[:, off + bB:off + 512], vaug[:, kB, :], pt[:, 1, bB:512],
                    start=False, stop=last, skip_group_check=True,
                )

            def chunk_slots(j):
                s = []
                for jj in range(j):
                    for m in (0, 1):
                        for rng in (0, 1):
                            s.append(("slot", dict(
                                j=j, kA=4 * jj + 2 * m, kB=4 * jj + 2 * m + 1,
                                c0=0, rng=rng, diag=False)))
                for m in (0, 1):
                    for rng in (0, 1):
                        s.append(("slot", dict(
                            j=j, kA=4 * j + 2 * m, kB=4 * j + 2 * m + 1,
                            c0=256 * m, rng=rng, diag=True,
                            last=(m == 1 and rng == 1))))
                return s

            c0s = chunk_slots(0)
            actions = [("proj_own", 0), ("proj_peer", 4)]
            actions += c0s[:1] + [("dma1", None)] + c0s[1:]
            actions += [("out", 0), ("warm", (0, 8))]
            actions += [("proj_own", 1), ("proj_peer", 5), ("dma2", None)]
            actions += chunk_slots(1) + [("out", 1), ("warm", (1, 4))]
            actions += [("proj_own", 2), ("proj_peer", 6),
                        ("proj_own", 3), ("proj_peer", 7)]
            c2, c3 = chunk_slots(2), chunk_slots(3)
            mix = []
            while c2 or c3:
                if c2:
                    mix.append(c2.pop(0))
                if c3:
                    mix.append(c3.pop(0))
                if c3:
                    mix.append(c3.pop(0))
            for a in mix:
                actions.append(a)
                if a[1].get("last") and a[1]["j"] == 2:
                    actions.append(("out", 2))
            actions.append(("out", 3))

            pending = None

            def flush():
                nonlocal pending
                if pending is not None:
                    a, pt = pending
                    pv(a["j"], a["kA"], a["kB"], pt, a["c0"], a["rng"],
                       a["diag"], a.get("last", False))
                    pending = None

            for kind, arg in actions:
                if kind == "proj_own":
                    proj_own(arg)
                elif kind == "proj_peer":
                    proj_peer(arg)
                elif kind == "dma1":
                    # Second DMA wave: gated behind chunk-0's mask ops in
                    # the gpsimd FIFO, so it starts only once the first
                    # panels have landed and been consumed.
                    nc.gpsimd.dma_start(xts[1][:], xt_d[1])
                    nc.gpsimd.dma_start(xts[2][:], xt_d[2])
                elif kind == "dma2":
                    # Third wave, behind chunk-0 exps in the scalar FIFO.
                    nc.scalar.dma_start(xts[3][:], xt_d[3])
                    nc.scalar.dma_start(xts[7][:], xt_d[7])
                elif kind == "warm":
                    # Keep-warm fillers bridge DMA-starved PE windows so the
                    # HAM clock gate stays at 8/8. They write the dead rows
                    # 96-127 of the retired chunk accumulator (never read;
                    # later reuse start=True-clears them).
                    jw, n = arg
                    for _ in range(n):
                        nc.tensor.matmul(
                            ot[jw][96:128, 0:512], scr[:, 0:32], scr[:],
                            start=True, stop=True, tile_position=(0, 96),
                            skip_group_check=True,
                        )
                elif kind == "slot":
                    if arg["j"] not in ot:
                        ot[arg["j"]] = psO.tile([128, 1024], F32, tag="ot", name="ot")
                    pt = scores_exp(arg["j"], arg["kA"], arg["kB"],
                                    arg["c0"], arg["rng"], arg["diag"])
                    flush()
                    pending = (arg, pt)
                else:  # out: partials via an SBUF bounce
                    flush()
                    osb = osbp.tile([65, 1024], F32, tag="osb")
                    nc.vector.tensor_copy(osb[:], ot[arg][0:65, :])
                    nc.sync.dma_start(out_d.ap()[arg], osb[:])
                    # peer-panel waves ride the sync FIFO behind this
                    # dependency-gated output DMA
                    if arg == 0:
                        nc.sync.dma_start(xts[5][:], xt_d[5])
                    elif arg == 1:
                        nc.sync.dma_start(xts[6][:], xt_d[6])

    nc.compile()
    return nc


def _own_blocks(j, r):
    """Global 128-row key-block positions (within chunk j) owned by
    parity r, in local order."""
    q = (j + r) % 2
    return [q + 2 * i for i in range(4)]


def make_inputs(x, Wk, Wv):
    """Build the 8 per-core input maps (pure layout work)."""
    bf16 = ml_dtypes.bfloat16
    wkv = np.concatenate([Wk, Wv], axis=1)            # [1024, 128]
    wkv_t = np.ascontiguousarray(
        wkv.reshape(NCHI, 128, 128).transpose(1, 0, 2)
    ).astype(bf16)  # [cp, chi, m]
    wkk = np.concatenate([Wk, Wk], axis=1)
    wkk_t = np.ascontiguousarray(
        wkk.reshape(NCHI, 128, 128).transpose(1, 0, 2)
    ).astype(bf16)

    eyeb = np.zeros((128, 65), dtype=np.float32)
    eyeb[64:128, 0:64] = np.eye(64)
    eyeb[:, 64] = 1.0

    kk = np.arange(128)[:, None]                      # key row within block
    cc = np.arange(256)[None, :]                      # window column

    # msk[*, 0]: own-range diag mask. A: tril then ones; B: zeros then tril.
    gm2 = np.empty((128, 2, 256), dtype=np.float32)
    gm2[:, 0, :] = np.concatenate(
        [(cc[:, 0:128] >= kk), np.ones((128, 128))], axis=1)
    gm2[:, 1, :] = np.concatenate(
        [np.zeros((128, 128)), (cc[:, 0:128] >= kk)], axis=1)

    def peer_mask(qpar):
        pm = np.ones((128, 2, 256), dtype=np.float32)
        pm[:, 1, 0:128] = 0.0  # B's never-written prefix stays inert
        if qpar == 1:
            pm[:, 0, 0:128] = 0.0
            pm[:, 1, :] = 0.0
        return pm

    in_maps = []
    for c in range(8):
        b, r = c % 4, c // 4

        xT = np.ascontiguousarray(x[b].T)             # [1024, 4096]
        xr = xT.reshape(NCHI, 128, T)                 # [chi, cp, t]
        xt = np.empty((NPAN, 128, NCHI, PAN), dtype=bf16)
        for p in range(NPAN):
            j, rr = (p, r) if p < 4 else (p - 4, 1 - r)
            rows = np.concatenate([
                np.arange(j * CHUNK + m * 128, j * CHUNK + m * 128 + 128)
                for m in _own_blocks(j, rr)
            ])
            xt[p] = xr[:, :, rows].transpose(1, 0, 2)

        cst = np.empty((128, CSTW), dtype=bf16)
        cst[:, 0:CW_WKV] = wkv_t.reshape(128, -1)
        cst[:, O_EYE:O_EYE + CW_EYE] = eyeb.astype(bf16)
        cst[:, O_WKK:O_WKK + CW_WKK] = wkk_t.reshape(128, -1)
        mtab = np.stack([gm2, peer_mask(r % 2), peer_mask((1 + r) % 2)], axis=1)
        cst[:, O_MSK:O_MSK + CW_MSK] = mtab.reshape(128, -1).astype(bf16)

        in_maps.append({"xt": xt, "cst": cst})
    return in_maps


_NC = None


def get_nc():
    global _NC
    if _NC is None:
        _NC = build_nc()
    return _NC


def kernel(x, Wk, Wv):
    x = np.asarray(x, dtype=np.float32)
    Wk = np.asarray(Wk, dtype=np.float32)
    Wv = np.asarray(Wv, dtype=np.float32)
    nc = get_nc()
    in_maps = make_inputs(x, Wk, Wv)
    res = run_bass_kernel_spmd(nc, in_maps, list(range(8)))

    out = np.empty((B, T, H), dtype=np.float32)
    for b in range(4):
        p0 = res.results[b]["out"].astype(np.float64)      # parity 0
        p1 = res.results[b + 4]["out"].astype(np.float64)  # parity 1
        for j in range(NCK):
            tot = np.zeros((65, 1024), dtype=np.float64)
            for r, part in ((0, p0[j]), (1, p1[j])):
                blocks = _own_blocks(j, r) + _own_blocks(j, 1 - r)
                glob = np.empty((65, 1024), dtype=np.float64)
                for k, m in enumerate(blocks):
                    glob[:, m * 128:(m + 1) * 128] = part[:, k * 128:(k + 1) * 128]
                tot += glob
            out[b, j * CHUNK:(j + 1) * CHUNK] = (tot[0:64] / tot[64]).T
    return out


# revision 28
# speedup vs baseline: 1.0557x; 1.0423x over previous
"""Causal single-head attention (Q==K source bug faithful) on 8 TRN2 NeuronCores.

Problem: x [4, 4096, 1024], Wk/Wv [1024, 64];
  k = q = x@Wk; scores = q k^T / 8, causal softmax, out = weights @ (x@Wv).

v5 strategy — balanced key-split with HOST-side softmax combine:
  - 8 cores = 4 batches x 2 "parities". Both cores of a batch project
    the full K (Q==K needs all queries anyway), but each core runs the
    attention for only HALF the key blocks: within each 1024-row key
    chunk j, core parity r owns the four 128-row key blocks at
    positions (j+r)%2 + {0,2,4,6}. This splits the causal triangle
    EXACTLY in half with a uniform 40-slot program per core (24 full
    off-diagonal slots + 16 trimmed diagonal slots) — no wasted
    compute, no cross-core traffic.
  - Each core emits per-chunk PARTIAL softmax accumulators
    [65, 1024] = (V|1)^T @ exp(S^T): rows 0..64 partial numerator^T,
    row 64 partial denominator, DMA'd straight from PSUM. The host
    adds the two cores' partials and divides — no epilogue transposes,
    no reciprocal, ~1us tail. (No max-subtraction needed: scores <=
    ~16, so partial sums stay well inside f32.)
  - Per-core data, uniform program: the host packs that core's own
    key blocks as x^T panels 0..3 and the peer's as panels 4..7, so
    all slot addressing (lhsT = own K^T block, Q = [own half | peer
    half] of a chunk) is core-independent; causal masks for the
    diagonal slots are a host-built table indexed by (chunk parity,
    block index).
  - Engine use: scores^T [keys, queries] on TensorE (bf16), exp on
    ScalarE (scale=1/8 fused) from PSUM, P@V accumulate on TensorE
    into the partial PSUM; software-pipelined one slot ahead so
    TensorE never waits on ScalarE; all panel DMAs issued upfront
    (panel-major contiguous layout, bf16).
"""
import numpy as np
import ml_dtypes

import concourse.bass as bass
import concourse.mybir as mybir
from concourse import bacc, tile
from concourse.bass_utils import run_bass_kernel_spmd

F32 = mybir.dt.float32
BF16 = mybir.dt.bfloat16
EXP = mybir.ActivationFunctionType.Exp

B, T, C, H = 4, 4096, 1024, 64
NCHI = C // 128          # 8 contraction blocks
NPAN = 8                 # panels (0..3 own, 4..7 peer), 512 rows each
PAN = 512
CHUNK = 1024             # queries per chunk
NCK = T // CHUNK         # 4 chunks


def build_nc():
    nc = bacc.Bacc("TRN2", target_bir_lowering=False, debug=False, num_devices=8)

    xt_d = nc.declare_dram_parameter("xt", [NPAN, 128, NCHI, PAN], BF16, isOutput=False)
    wkv_d = nc.declare_dram_parameter("wkv", [128, NCHI, 128], BF16, isOutput=False)
    gm_d = nc.declare_dram_parameter("gm", [128, PAN], BF16, isOutput=False)
    mb_d = nc.declare_dram_parameter("mb", [128, 2, 128], BF16, isOutput=False)
    eyeb_d = nc.declare_dram_parameter("eyeb", [128, 65], BF16, isOutput=False)
    out_d = nc.declare_dram_parameter("out", [NCK, 65, 1024], F32, isOutput=True)

    with tile.TileContext(nc) as tc:
        with (
            tc.tile_pool(name="const", bufs=1) as const,
            tc.tile_pool(name="xt", bufs=NPAN) as xtp,
            tc.tile_pool(name="kv", bufs=3) as kvp,
            tc.tile_pool(name="pt", bufs=8) as ptp,
            tc.tile_pool(name="osb", bufs=2) as osbp,
            tc.tile_pool(name="psA", bufs=2, space="PSUM") as psA,
            tc.tile_pool(name="psO", bufs=2, space="PSUM") as psO,
        ):
            wkv = const.tile([128, NCHI, 128], BF16, tag="wkv")
            gm = const.tile([128, PAN], BF16, tag="gm")
            mb = const.tile([128, 2, 128], BF16, tag="mb")
            eyeb = const.tile([128, 65], BF16, tag="eyeb")
            kt = const.tile([64, T], BF16, tag="kt")     # [own 2048 | peer 2048]
            vaug = const.tile([128, 16, 65], BF16, tag="vaug")  # own V|1 per kb

            scr = const.tile([64, 512], BF16, tag="scr")
            nc.gpsimd.memset(scr[:], 0)
            wps = psA.tile([128, 1024], F32, tag="ps", name="warm")
            for _ in range(12):
                nc.tensor.matmul(
                    wps[0:65, 0:512], scr[:, 0:65], scr[:],
                    start=True, stop=True,
                )
            nc.gpsimd.dma_start(wkv[:], wkv_d[:])
            nc.sync.dma_start(gm[:], gm_d[:])
            nc.sync.dma_start(mb[:], mb_d[:])
            nc.sync.dma_start(eyeb[:], eyeb_d[:])
            # ones column of every V|1 block
            nc.vector.tensor_copy(
                vaug[:, :, 64:65],
                eyeb[:, 64:65].unsqueeze(1).broadcast_to([128, 16, 1]),
            )

            for _ in range(2):
                z = psA.tile([128, 1024], F32, tag="ps", name="z")
                nc.scalar.memzero(z[:])

            # Panel DMAs in staggered waves: the DMA engines round-robin
            # across ALL pending transfers, so panels 0/4 are issued alone
            # first (split in halves); later panels are issued mid-program
            # behind dependency-gated instructions in each engine's FIFO.
            xts = []
            for p in range(NPAN):
                xt = xtp.tile([128, NCHI, PAN], BF16, tag="xt", name=f"xt{p}")
                xts.append(xt)
            nc.gpsimd.dma_start(xts[0][:, 0:4, :], xt_d[0][:, 0:4, :])
            nc.gpsimd.dma_start(xts[0][:, 4:8, :], xt_d[0][:, 4:8, :])
            nc.sync.dma_start(xts[4][:, 0:4, :], xt_d[4][:, 0:4, :])
            nc.sync.dma_start(xts[4][:, 4:8, :], xt_d[4][:, 4:8, :])

            def proj_panel(p):
                """Project K^T (and V^T for own panels) of panel p."""
                xt = xts[p]
                pj = psA.tile([128, 1024], F32, tag="ps")
                kv_ps = pj[:, 0:PAN]
                if p >= 4:  # peer panel: only K^T needed (queries), m=64
                    for ci in range(NCHI):
                        nc.tensor.matmul(
                            pj[0:64, 0:PAN], wkv[:, ci, 0:64], xt[:, ci, :],
                            start=(ci == 0), stop=(ci == NCHI - 1),
                        )
                    nc.vector.tensor_copy(kt[:, p * PAN:(p + 1) * PAN], pj[0:64, 0:PAN])
                    return
                for ci in range(NCHI):
                    nc.tensor.matmul(
                        kv_ps[:], wkv[:, ci, :], xt[:, ci, :],
                        start=(ci == 0), stop=(ci == NCHI - 1),
                    )
                kvsb = kvp.tile([128, PAN], BF16, tag="kv")
                nc.vector.tensor_copy(kvsb[:], kv_ps[:])
                nc.vector.tensor_copy(kt[:, p * PAN:(p + 1) * PAN], kvsb[0:64, :])
                v_ps = pj[:, PAN:PAN + 128].bitcast(BF16).rearrange(
                    "p (a b) -> p a b", a=4)
                for tb in range(4):
                    nc.tensor.transpose(
                        v_ps[:, tb, :], kvsb[64:128, tb * 128:(tb + 1) * 128],
                        eyeb[64:128, 0:64],
                    )
                nc.vector.tensor_copy(vaug[:, 4 * p:4 * p + 4, 0:64], v_ps[:])

            ot = {}  # live chunk accumulators [65, 1024]

            def scores_exp(j, lkb, i=None):
                """Slot front half. lkb = own local key block (0..15).
                i = None: off-diagonal (full). i = 0..3: diagonal block
                index; both halves trimmed to suffix [128i:]; the exp
                spans [128i:1024] (the stale gap [512:512+128i] is
                pre-zeroed/finite and never read by PV); gm masks the
                own triangle, mb zeroes the first peer block when this
                chunk parity makes it non-causal."""
                c0 = 0 if i is None else 128 * i
                s_ps = psA.tile([128, 1024], F32, tag="ps")
                lhsT = kt[:, lkb * 128:(lkb + 1) * 128]
                nc.tensor.matmul(
                    s_ps[:, c0:512],
                    lhsT, kt[:, j * PAN + c0:(j + 1) * PAN],
                    start=True, stop=True,
                )
                nc.tensor.matmul(
                    s_ps[:, 512 + c0:1024],
                    lhsT, kt[:, 2048 + j * PAN + c0:2048 + (j + 1) * PAN],
                    start=True, stop=True,
                )
                pt = ptp.tile([128, 1024], BF16, tag="pt")
                nc.scalar.activation(pt[:, c0:1024], s_ps[:, c0:1024], EXP, scale=0.125)
                if i is not None:
                    nc.vector.tensor_mul(
                        pt[:, c0:512], pt[:, c0:512], gm[:, 0:512 - c0]
                    )
                    nc.vector.tensor_mul(
                        pt[:, 512 + c0:512 + c0 + 128],
                        pt[:, 512 + c0:512 + c0 + 128], mb[:, j % 2, :]
                    )
                return pt

            def pv(j, lkb, pt, i=None, first=False, last=False):
                c0 = 0 if i is None else 128 * i
                if first:
                    ot[j] = psO.tile([65, 1024], F32, tag="ot", name="ot")
                acc = ot[j]
                nc.tensor.matmul(
                    acc[:, c0:512], vaug[:, lkb, :], pt[:, c0:512],
                    start=first, stop=last,
                )
                nc.tensor.matmul(
                    acc[:, 512 + c0:1024], vaug[:, lkb, :], pt[:, 512 + c0:1024],
                    start=first, stop=last,
                )

            # ---- schedule: per chunk j: off-diagonal kbs then diagonal ----
            def chunk_slots(j):
                s = []
                for lkb in range(4 * j):
                    s.append(("slot", dict(j=j, lkb=lkb, first=(lkb == 0))))
                for i in range(4):
                    s.append(("slot", dict(
                        j=j, lkb=4 * j + i, i=i,
                        first=(j == 0 and i == 0), last=(i == 3))))
                return s

            actions = [("proj", 0), ("proj", 4), ("dma1", None)]
            actions += chunk_slots(0) + [("out", 0)]
            actions += [("proj", 1), ("proj", 5)]
            actions += chunk_slots(1) + [("out", 1)]
            actions += [("proj", 2), ("proj", 6), ("proj", 3), ("proj", 7)]
            c2, c3 = chunk_slots(2), chunk_slots(3)
            mix = []
            while c2 or c3:  # interleave 2:3 to even out the tail
                if c2:
                    mix.append(c2.pop(0))
                if c3:
                    mix.append(c3.pop(0))
                if c3:
                    mix.append(c3.pop(0))
            for a in mix:
                actions.append(a)
                if a[1].get("last") and a[1]["j"] == 2:
                    actions.append(("out", 2))
            actions.append(("out", 3))

            pending = None

            def flush():
                nonlocal pending
                if pending is not None:
                    a, pt = pending
                    pv(a["j"], a["lkb"], pt, i=a.get("i"),
                       first=a.get("first", False), last=a.get("last", False))
                    pending = None

            for kind, arg in actions:
                if kind == "proj":
                    proj_panel(arg)
                elif kind == "dma1":
                    # gate: waits (via Tile dep) until proj-0's kt write, so
                    # the wave-2 issues below it in the gpsimd FIFO can't
                    # steal DMA bandwidth from panels 0/4
                    nc.gpsimd.tensor_copy(scr[:, 0:1], kt[:, 0:1])
                    nc.gpsimd.dma_start(xts[1][:], xt_d[1])
                    nc.gpsimd.dma_start(xts[2][:], xt_d[2])
                    nc.gpsimd.dma_start(xts[3][:], xt_d[3])
                elif kind == "slot":
                    pt = scores_exp(arg["j"], arg["lkb"], i=arg.get("i"))
                    flush()
                    pending = (arg, pt)
                else:  # out: partials via an SBUF bounce
                    flush()
                    osb = osbp.tile([65, 1024], F32, tag="osb")
                    nc.vector.tensor_copy(osb[:], ot[arg][:])
                    nc.sync.dma_start(out_d.ap()[arg], osb[:])
                    if arg == 0:
                        nc.sync.dma_start(xts[5][:], xt_d[5])
                        nc.sync.dma_start(xts[6][:], xt_d[6])
                    elif arg == 1:
                        nc.sync.dma_start(xts[7][:], xt_d[7])

    nc.compile()
    return nc


def _own_blocks(j, r):
    """Global 128-row key-block positions (within chunk j) owned by
    parity r, in local order."""
    q = (j + r) % 2
    return [q + 2 * i for i in range(4)]


def make_inputs(x, Wk, Wv):
    """Build the 8 per-core input maps (pure layout work)."""
    bf16 = ml_dtypes.bfloat16
    wkv = np.concatenate([Wk, Wv], axis=1)            # [1024, 128]
    wkv_t = np.ascontiguousarray(
        wkv.reshape(NCHI, 128, 128).transpose(1, 0, 2)
    ).astype(bf16)  # [cp, chi, m]

    eyeb = np.zeros((128, 65), dtype=np.float32)
    eyeb[64:128, 0:64] = np.eye(64)
    eyeb[:, 64] = 1.0
    eyeb = eyeb.astype(bf16)

    pp = np.arange(128)[:, None]                      # key row within block
    cc = np.arange(1024)[None, :]                     # query column

    in_maps = []
    for c in range(8):
        b, r = c % 4, c // 4

        # x^T panels: own rows (panels 0..3) then peer rows (4..7)
        xT = np.ascontiguousarray(x[b].T)             # [1024, 4096]
        xr = xT.reshape(NCHI, 128, T)                 # [chi, cp, t]
        xt = np.empty((NPAN, 128, NCHI, PAN), dtype=bf16)
        for p in range(NPAN):
            j, rr = (p, r) if p < 4 else (p - 4, 1 - r)
            rows = np.concatenate([
                np.arange(j * CHUNK + m * 128, j * CHUNK + m * 128 + 128)
                for m in _own_blocks(j, rr)
            ])
            xt[p] = xr[:, :, rows].transpose(1, 0, 2)

        # Own-half triangle mask (block 0 triu, rest ones) and the
        # per-chunk-parity first-peer-block mask (zero iff own parity 1).
        gmm = np.ones((128, PAN), dtype=np.float32)
        gmm[:, 0:128] = (cc[:, 0:128] >= pp).astype(np.float32)
        mbm = np.empty((128, 2, 128), dtype=np.float32)
        for jp in range(2):
            mbm[:, jp, :] = 0.0 if (jp + r) % 2 == 1 else 1.0

        in_maps.append({"xt": xt, "wkv": wkv_t, "gm": gmm.astype(bf16),
                        "mb": mbm.astype(bf16), "eyeb": eyeb})
    return in_maps


_NC = None


def get_nc():
    global _NC
    if _NC is None:
        _NC = build_nc()
    return _NC


def kernel(x, Wk, Wv):
    x = np.asarray(x, dtype=np.float32)
    Wk = np.asarray(Wk, dtype=np.float32)
    Wv = np.asarray(Wv, dtype=np.float32)
    nc = get_nc()
    in_maps = make_inputs(x, Wk, Wv)
    res = run_bass_kernel_spmd(nc, in_maps, list(range(8)))

    out = np.empty((B, T, H), dtype=np.float32)
    for b in range(4):
        p0 = res.results[b]["out"].astype(np.float64)      # parity 0
        p1 = res.results[b + 4]["out"].astype(np.float64)  # parity 1
        for j in range(NCK):
            tot = np.zeros((65, 1024), dtype=np.float64)
            for r, part in ((0, p0[j]), (1, p1[j])):
                blocks = _own_blocks(j, r) + _own_blocks(j, 1 - r)
                glob = np.empty((65, 1024), dtype=np.float64)
                for k, m in enumerate(blocks):
                    glob[:, m * 128:(m + 1) * 128] = part[:, k * 128:(k + 1) * 128]
                tot += glob
            out[b, j * CHUNK:(j + 1) * CHUNK] = (tot[0:64] / tot[64]).T
    return out



# revision 30
# speedup vs baseline: 1.0894x; 1.0320x over previous
"""Causal single-head attention (Q==K source bug faithful) on 8 TRN2 NeuronCores.

Problem: x [4, 4096, 1024], Wk/Wv [1024, 64];
  k = q = x@Wk; scores = q k^T / 8, causal softmax, out = weights @ (x@Wv).

v5 strategy — balanced key-split with HOST-side softmax combine:
  - 8 cores = 4 batches x 2 "parities". Both cores of a batch project
    the full K (Q==K needs all queries anyway), but each core runs the
    attention for only HALF the key blocks: within each 1024-row key
    chunk j, core parity r owns the four 128-row key blocks at
    positions (j+r)%2 + {0,2,4,6}. This splits the causal triangle
    EXACTLY in half with a uniform 40-slot program per core (24 full
    off-diagonal slots + 16 trimmed diagonal slots) — no wasted
    compute, no cross-core traffic.
  - Each core emits per-chunk PARTIAL softmax accumulators
    [65, 1024] = (V|1)^T @ exp(S^T): rows 0..64 partial numerator^T,
    row 64 partial denominator, DMA'd straight from PSUM. The host
    adds the two cores' partials and divides — no epilogue transposes,
    no reciprocal, ~1us tail. (No max-subtraction needed: scores <=
    ~16, so partial sums stay well inside f32.)
  - Per-core data, uniform program: the host packs that core's own
    key blocks as x^T panels 0..3 and the peer's as panels 4..7, so
    all slot addressing (lhsT = own K^T block, Q = [own half | peer
    half] of a chunk) is core-independent; causal masks for the
    diagonal slots are a host-built table indexed by (chunk parity,
    block index).
  - Engine use: scores^T [keys, queries] on TensorE (bf16), exp on
    ScalarE (scale=1/8 fused) from PSUM, P@V accumulate on TensorE
    into the partial PSUM; software-pipelined one slot ahead so
    TensorE never waits on ScalarE; all panel DMAs issued upfront
    (panel-major contiguous layout, bf16).
"""
import numpy as np
import ml_dtypes

import concourse.bass as bass
import concourse.mybir as mybir
from concourse import bacc, tile
from concourse.bass_utils import run_bass_kernel_spmd

F32 = mybir.dt.float32
BF16 = mybir.dt.bfloat16
EXP = mybir.ActivationFunctionType.Exp

B, T, C, H = 4, 4096, 1024, 64
NCHI = C // 128          # 8 contraction blocks
NPAN = 8                 # panels (0..3 own, 4..7 peer), 512 rows each
PAN = 512
CHUNK = 1024             # queries per chunk
NCK = T // CHUNK         # 4 chunks


def build_nc():
    nc = bacc.Bacc("TRN2", target_bir_lowering=False, debug=False, num_devices=8)

    xt_d = nc.declare_dram_parameter("xt", [NPAN, 128, NCHI, PAN], BF16, isOutput=False)
    wkv_d = nc.declare_dram_parameter("wkv", [128, NCHI, 128], BF16, isOutput=False)
    wkk_d = nc.declare_dram_parameter("wkk", [128, NCHI, 128], BF16, isOutput=False)
    gm_d = nc.declare_dram_parameter("gm", [128, PAN], BF16, isOutput=False)
    mb_d = nc.declare_dram_parameter("mb", [128, 2, 128], BF16, isOutput=False)
    eyeb_d = nc.declare_dram_parameter("eyeb", [128, 65], BF16, isOutput=False)
    out_d = nc.declare_dram_parameter("out", [NCK, 65, 1024], F32, isOutput=True)

    with tile.TileContext(nc) as tc:
        with (
            tc.tile_pool(name="const", bufs=1) as const,
            tc.tile_pool(name="xt", bufs=NPAN) as xtp,
            tc.tile_pool(name="kv", bufs=3) as kvp,
            tc.tile_pool(name="pt", bufs=8) as ptp,
            tc.tile_pool(name="osb", bufs=2) as osbp,
            tc.tile_pool(name="psA", bufs=2, space="PSUM") as psA,
            tc.tile_pool(name="psO", bufs=2, space="PSUM") as psO,
        ):
            wkv = const.tile([128, NCHI, 128], BF16, tag="wkv")
            wkk = const.tile([128, NCHI, 128], BF16, tag="wkk")
            gm = const.tile([128, PAN], BF16, tag="gm")
            mb = const.tile([128, 2, 128], BF16, tag="mb")
            eyeb = const.tile([128, 65], BF16, tag="eyeb")
            kt = const.tile([128, T], BF16, tag="kt")    # [own 2048 | peer 2048],
            # K^T on rows 0-63, duplicate on 64-127 for row-packed slots
            vaug = const.tile([128, 16, 65], BF16, tag="vaug")  # own V|1 per kb

            scr = const.tile([64, 512], BF16, tag="scr")
            nc.gpsimd.memset(scr[:], 0)
            wps = psA.tile([128, 1024], F32, tag="ps", name="warm")
            for _ in range(12):
                nc.tensor.matmul(
                    wps[0:65, 0:512], scr[:, 0:65], scr[:],
                    start=True, stop=True,
                )
            nc.gpsimd.dma_start(wkv[:], wkv_d[:])
            nc.gpsimd.dma_start(wkk[:], wkk_d[:])
            nc.sync.dma_start(gm[:], gm_d[:])
            nc.sync.dma_start(mb[:], mb_d[:])
            nc.sync.dma_start(eyeb[:], eyeb_d[:])
            # ones column of every V|1 block
            nc.vector.tensor_copy(
                vaug[:, :, 64:65],
                eyeb[:, 64:65].unsqueeze(1).broadcast_to([128, 16, 1]),
            )

            for _ in range(2):
                z = psA.tile([128, 1024], F32, tag="ps", name="z")
                nc.scalar.memzero(z[:])

            # Panel DMAs in staggered waves: the DMA engines round-robin
            # across ALL pending transfers, so panels 0/4 are issued alone
            # first (split in halves); later panels are issued mid-program
            # behind dependency-gated instructions in each engine's FIFO.
            xts = []
            for p in range(NPAN):
                xt = xtp.tile([128, NCHI, PAN], BF16, tag="xt", name=f"xt{p}")
                xts.append(xt)
            nc.gpsimd.dma_start(xts[0][:, 0:4, :], xt_d[0][:, 0:4, :])
            nc.gpsimd.dma_start(xts[0][:, 4:8, :], xt_d[0][:, 4:8, :])
            nc.sync.dma_start(xts[4][:, 0:4, :], xt_d[4][:, 0:4, :])
            nc.sync.dma_start(xts[4][:, 4:8, :], xt_d[4][:, 4:8, :])

            def proj_panel(p):
                """Project K^T (and V^T for own panels) of panel p."""
                xt = xts[p]
                pj = psA.tile([128, 1024], F32, tag="ps")
                kv_ps = pj[:, 0:PAN]
                if p >= 4:  # peer panel: [Wk|Wk] -> K^T duplicated, m=128
                    for ci in range(NCHI):
                        nc.tensor.matmul(
                            pj[:, 0:PAN], wkk[:, ci, :], xt[:, ci, :],
                            start=(ci == 0), stop=(ci == NCHI - 1),
                        )
                    nc.vector.tensor_copy(kt[:, p * PAN:(p + 1) * PAN], pj[:, 0:PAN])
                    return
                for ci in range(NCHI):
                    nc.tensor.matmul(
                        kv_ps[:], wkv[:, ci, :], xt[:, ci, :],
                        start=(ci == 0), stop=(ci == NCHI - 1),
                    )
                kvsb = kvp.tile([128, PAN], BF16, tag="kv")
                nc.vector.tensor_copy(kvsb[:], kv_ps[:])
                rng = slice(p * PAN, (p + 1) * PAN)
                nc.vector.tensor_copy(kt[0:64, rng], kvsb[0:64, :])
                # duplicate own K^T onto rows 64-127: panel 0 via the empty
                # scalar HW queue, later panels ride gpsimd after wave 2
                dq = nc.scalar if p == 0 else nc.gpsimd
                dq.dma_start(kt[64:128, rng], kt[0:64, rng])
                v_ps = pj[:, PAN:PAN + 128].bitcast(BF16).rearrange(
                    "p (a b) -> p a b", a=4)
                for tb in range(4):
                    nc.tensor.transpose(
                        v_ps[:, tb, :], kvsb[64:128, tb * 128:(tb + 1) * 128],
                        eyeb[64:128, 0:64],
                    )
                nc.vector.tensor_copy(vaug[:, 4 * p:4 * p + 4, 0:64], v_ps[:])

            ot = {}  # live chunk accumulators [65, 1024]

            def scores_exp(j, lkb, i=None):
                """Slot front half. lkb = own local key block (0..15).
                i = None: off-diagonal (full). i = 0..3: diagonal block
                index; both halves trimmed to suffix [128i:]; the exp
                spans [128i:1024] (the stale gap [512:512+128i] is
                pre-zeroed/finite and never read by PV); gm masks the
                own triangle, mb zeroes the first peer block when this
                chunk parity makes it non-causal."""
                c0 = 0 if i is None else 128 * i
                s_ps = psA.tile([128, 1024], F32, tag="ps")
                lo = kt[0:64, :]
                # chunks > 0: peer-half matmul runs on PE rows 64-127
                # concurrently with the own-half on rows 0-63 (both use the
                # same key block; kt rows 64-127 hold the duplicate)
                hi = kt[64:128, :] if j != 0 else lo
                tp = (64, 0) if j != 0 else (0, 0)
                nc.tensor.matmul(
                    s_ps[:, c0:512],
                    lo[:, lkb * 128:(lkb + 1) * 128],
                    lo[:, j * PAN + c0:(j + 1) * PAN],
                    start=True, stop=True, tile_position=(0, 0),
                )
                nc.tensor.matmul(
                    s_ps[:, 512 + c0:1024],
                    hi[:, lkb * 128:(lkb + 1) * 128],
                    hi[:, 2048 + j * PAN + c0:2048 + (j + 1) * PAN],
                    start=True, stop=True, tile_position=tp,
                )
                pt = ptp.tile([128, 1024], BF16, tag="pt")
                nc.scalar.activation(pt[:, c0:1024], s_ps[:, c0:1024], EXP, scale=0.125)
                if i is not None:
                    nc.vector.tensor_mul(
                        pt[:, c0:512], pt[:, c0:512], gm[:, 0:512 - c0]
                    )
                    nc.vector.tensor_mul(
                        pt[:, 512 + c0:512 + c0 + 128],
                        pt[:, 512 + c0:512 + c0 + 128], mb[:, j % 2, :]
                    )
                return pt

            def pv(j, lkb, pt, i=None, first=False, last=False):
                c0 = 0 if i is None else 128 * i
                if first:
                    ot[j] = psO.tile([65, 1024], F32, tag="ot", name="ot")
                acc = ot[j]
                nc.tensor.matmul(
                    acc[:, c0:512], vaug[:, lkb, :], pt[:, c0:512],
                    start=first, stop=last,
                )
                nc.tensor.matmul(
                    acc[:, 512 + c0:1024], vaug[:, lkb, :], pt[:, 512 + c0:1024],
                    start=first, stop=last,
                )

            # ---- schedule: per chunk j: off-diagonal kbs then diagonal ----
            def chunk_slots(j):
                s = []
                for lkb in range(4 * j):
                    s.append(("slot", dict(j=j, lkb=lkb, first=(lkb == 0))))
                for i in range(4):
                    s.append(("slot", dict(
                        j=j, lkb=4 * j + i, i=i,
                        first=(j == 0 and i == 0), last=(i == 3))))
                return s

            c0s = chunk_slots(0)
            actions = [("proj", 0), ("proj", 4), ("dma1", None)]
            actions += c0s[:1] + [("dma3", None)] + c0s[1:] + [("out", 0)]
            actions += [("proj", 1), ("proj", 5)]
            actions += chunk_slots(1) + [("out", 1)]
            actions += [("proj", 2), ("proj", 6), ("proj", 3), ("proj", 7)]
            c2, c3 = chunk_slots(2), chunk_slots(3)
            mix = []
            while c2 or c3:  # interleave 2:3 to even out the tail
                if c2:
                    mix.append(c2.pop(0))
                if c3:
                    mix.append(c3.pop(0))
                if c3:
                    mix.append(c3.pop(0))
            for a in mix:
                actions.append(a)
                if a[1].get("last") and a[1]["j"] == 2:
                    actions.append(("out", 2))
            actions.append(("out", 3))

            pending = None

            def flush():
                nonlocal pending
                if pending is not None:
                    a, pt = pending
                    pv(a["j"], a["lkb"], pt, i=a.get("i"),
                       first=a.get("first", False), last=a.get("last", False))
                    pending = None

            for kind, arg in actions:
                if kind == "proj":
                    proj_panel(arg)
                elif kind == "dma1":
                    # gate: waits (via Tile dep) until panel 0 has fully
                    # landed, so the wave-2 issues below it in the gpsimd
                    # FIFO can't steal DMA bandwidth from panels 0/4
                    nc.gpsimd.tensor_copy(scr[:, 0:1], xts[0][0:64, 7, 0:1])
                    nc.gpsimd.dma_start(xts[1][:], xt_d[1])
                    nc.gpsimd.dma_start(xts[2][:], xt_d[2])
                elif kind == "dma3":
                    # panels 5/7 ride the scalar FIFO behind chunk-0's first
                    # exp, i.e. they start once panels 0/4 are consumed
                    nc.scalar.dma_start(xts[5][:], xt_d[5])
                    nc.scalar.dma_start(xts[7][:], xt_d[7])
                elif kind == "slot":
                    pt = scores_exp(arg["j"], arg["lkb"], i=arg.get("i"))
                    flush()
                    pending = (arg, pt)
                else:  # out: partials via an SBUF bounce
                    flush()
                    osb = osbp.tile([65, 1024], F32, tag="osb")
                    nc.vector.tensor_copy(osb[:], ot[arg][:])
                    nc.sync.dma_start(out_d.ap()[arg], osb[:])
                    if arg == 0:
                        nc.sync.dma_start(xts[3][:], xt_d[3])
                        nc.sync.dma_start(xts[6][:], xt_d[6])

    nc.compile()
    return nc


def _own_blocks(j, r):
    """Global 128-row key-block positions (within chunk j) owned by
    parity r, in local order."""
    q = (j + r) % 2
    return [q + 2 * i for i in range(4)]


def make_inputs(x, Wk, Wv):
    """Build the 8 per-core input maps (pure layout work)."""
    bf16 = ml_dtypes.bfloat16
    wkv = np.concatenate([Wk, Wv], axis=1)            # [1024, 128]
    wkv_t = np.ascontiguousarray(
        wkv.reshape(NCHI, 128, 128).transpose(1, 0, 2)
    ).astype(bf16)  # [cp, chi, m]
    wkk = np.concatenate([Wk, Wk], axis=1)
    wkk_t = np.ascontiguousarray(
        wkk.reshape(NCHI, 128, 128).transpose(1, 0, 2)
    ).astype(bf16)

    eyeb = np.zeros((128, 65), dtype=np.float32)
    eyeb[64:128, 0:64] = np.eye(64)
    eyeb[:, 64] = 1.0
    eyeb = eyeb.astype(bf16)

    pp = np.arange(128)[:, None]                      # key row within block
    cc = np.arange(1024)[None, :]                     # query column

    in_maps = []
    for c in range(8):
        b, r = c % 4, c // 4

        # x^T panels: own rows (panels 0..3) then peer rows (4..7)
        xT = np.ascontiguousarray(x[b].T)             # [1024, 4096]
        xr = xT.reshape(NCHI, 128, T)                 # [chi, cp, t]
        xt = np.empty((NPAN, 128, NCHI, PAN), dtype=bf16)
        for p in range(NPAN):
            j, rr = (p, r) if p < 4 else (p - 4, 1 - r)
            rows = np.concatenate([
                np.arange(j * CHUNK + m * 128, j * CHUNK + m * 128 + 128)
                for m in _own_blocks(j, rr)
            ])
            xt[p] = xr[:, :, rows].transpose(1, 0, 2)

        # Own-half triangle mask (block 0 triu, rest ones) and the
        # per-chunk-parity first-peer-block mask (zero iff own parity 1).
        gmm = np.ones((128, PAN), dtype=np.float32)
        gmm[:, 0:128] = (cc[:, 0:128] >= pp).astype(np.float32)
        mbm = np.empty((128, 2, 128), dtype=np.float32)
        for jp in range(2):
            mbm[:, jp, :] = 0.0 if (jp + r) % 2 == 1 else 1.0

        in_maps.append({"xt": xt, "wkv": wkv_t, "wkk": wkk_t,
                        "gm": gmm.astype(bf16),
                        "mb": mbm.astype(bf16), "eyeb": eyeb})
    return in_maps


_NC = None


def get_nc():
    global _NC
    if _NC is None:
        _NC = build_nc()
    return _NC


def kernel(x, Wk, Wv):
    x = np.asarray(x, dtype=np.float32)
    Wk = np.asarray(Wk, dtype=np.float32)
    Wv = np.asarray(Wv, dtype=np.float32)
    nc = get_nc()
    in_maps = make_inputs(x, Wk, Wv)
    res = run_bass_kernel_spmd(nc, in_maps, list(range(8)))

    out = np.empty((B, T, H), dtype=np.float32)
    for b in range(4):
        p0 = res.results[b]["out"].astype(np.float64)      # parity 0
        p1 = res.results[b + 4]["out"].astype(np.float64)  # parity 1
        for j in range(NCK):
            tot = np.zeros((65, 1024), dtype=np.float64)
            for r, part in ((0, p0[j]), (1, p1[j])):
                blocks = _own_blocks(j, r) + _own_blocks(j, 1 - r)
                glob = np.empty((65, 1024), dtype=np.float64)
                for k, m in enumerate(blocks):
                    glob[:, m * 128:(m + 1) * 128] = part[:, k * 128:(k + 1) * 128]
                tot += glob
            out[b, j * CHUNK:(j + 1) * CHUNK] = (tot[0:64] / tot[64]).T
    return out

